# revision 39
# baseline (speedup 1.0000x reference)
"""Trainium2 Bass kernel: batched self-attention layer.

Per-batch attention (B=8, S=4096, D=128), data-parallel: one batch
element per NeuronCore across 8 cores.  Per core:

  Q = x @ Wq^T, K = x @ Wk^T, V = x @ Wv^T
  out = softmax(Q @ K^T) @ V          (unscaled logits)

Design (per core).  The kernel is a software pipeline around TWO exp
engines working the same PSUM score stream:

  - the ACT (scalar) engine computes exp groups with
    activation(Exp, bias=-SHIFT), SHIFT = 127*ln2 - 0.0397.
  - the DVE (vector) engine computes 2 of the 11 k-tile groups per
    q-chunk with a one-instruction Schraudolph exp:
      i16 = int16(max(s * 128/ln2, 0))  bitcast as bf16
    which is e^(s - 127*ln2) * (1 + eps(f)), eps in [0, +6.1%] a
    sawtooth in the fractional mantissa.  The ACT shift is offset by
    the mean ln-inflation (0.0397) so both streams carry the SAME
    effective scale; the residual +-3% per-weight sawtooth is
    softmax-noise, measured end-to-end at ~3e-3 output rel err.
    No clamp pass is needed: max(.,0) underflows cleanly to +0 and the
    i16 can neither wrap (max logit 125 -> 23083) nor form NaN bit
    patterns (needs >= 32640).
  - scores are folded:  Q K^T = x (Wq^T Wk) x^T.  M = Wq^T Wk is one
    128x128 matmul of the two NATURAL-layout weights, then
    aT = (x M)^T and scoresT[k, q] = xT_chunk.T @ aT.
  - x is PE-transposed once to xT [d=128 part, s=4096]; x tiles are
    tagged f32r and the transpose identity is BF16, putting the
    transpose matmuls (cost follows the MOVING operand = identity) at
    1 cyc/row instead of 2.
  - V-projection runs fully in bf16 (xTb = bf16 copy of xT, wvb =
    bf16 Wv^T): 128-wide moving tiles at 1 cyc/row, half the f32r
    cost (which needs 256-wide moving to avoid the 4x penalty).
  - PV uses exp tiles as the STATIONARY operand and [V | ones] as the
    bf16 moving operand, so the softmax denominator accumulates in
    PSUM as a free 129th output column; one accumulation group per
    2KB zero-region (per bank), as the hardware requires.
  - normalize = DVE reciprocal of column 128 + per-partition scalar
    multiply, then per-subtile DMA out.  A per-row scale cancels in
    this division, which is what makes the shared-shift scheme exact.
  - the final 512 queries run as two 256-wide half-units (one PV wave
    each) so the last unit's PV trails its exps directly.
  - PSUM budget: 6 banks score/exp double buffer + 2 banks phase-1
    (later reused as PV accumulators) = 8.
"""

import math
import sys

for _p in ("/opt/trn_rl_repo", "/root/.axon_site/_ro/trn_rl_repo"):
    if _p not in sys.path:
        sys.path.append(_p)

import numpy as np

import concourse.bass as bass
import concourse.bacc as bacc
import concourse.mybir as mybir
from concourse.bass_utils import run_bass_kernel_spmd
from concourse.masks import make_identity
from concourse.tile import TileContext

F32 = mybir.dt.float32
F32R = mybir.dt.float32r
BF16 = mybir.dt.bfloat16
I16 = mybir.dt.int16

B, S, D = 8, 4096, 128
P = 128
N_CORES = 8
# ACT softmax shift: the DVE bit-hack exp carries an implicit shift of
# 127*ln2 plus a mean ln-inflation of 0.039721 (Schraudolph sawtooth);
# ACT matches it so both streams mix seamlessly in one PV accumulation.
SCHRAUDOLPH_CENTER = 0.039721
SHIFT = 127.0 * math.log(2.0) - SCHRAUDOLPH_CENTER
C1 = 128.0 / math.log(2.0)  # 184.664965...
Q_CHUNK = 512
N_QCHUNKS = S // Q_CHUNK  # 8
N_KTILES = S // P  # 32
KT_PAIR = 2  # k-tiles per scores-psum/exp group

# exp groups handed to the DVE engine (by group index within a unit).
# Do NOT give DVE an early group: scores g+2 waits on exp g (PSUM
# rotation), which would splice DVE's queue latency into the PE score
# stream.  Overridable via env for scheduling sweeps.
import os as _os


def _env_groups(name, default):
    v = _os.environ.get(name)
    if not v:
        return default
    return tuple(int(t) for t in v.split(",") if t != "")


# k-tiles per scores/exp group: 3 -> 2-deep PSUM rotation (2x3 banks),
# 2 -> 3-deep rotation (3x2 banks); PV accumulators use the other 2 banks.
KT_GRP = int(_os.environ.get("ATT_KT_GRP", "2"))
N_KG = -(-N_KTILES // KT_GRP)  # 11 (grp3: 10x3+1x2) or 16 (grp2)
_DVE_DEFAULTS = {3: ((3, 8), (5,)), 2: ((6, 11, 14), (11, 14))}
DVE_FULL = _env_groups("ATT_DVE_FULL", _DVE_DEFAULTS[KT_GRP][0])
DVE_HALF = _env_groups("ATT_DVE_HALF", _DVE_DEFAULTS[KT_GRP][1])
X_RING = _os.environ.get("ATT_X_RING", "sync")


def build_attention_nc():
    nc = bacc.Bacc(None, target_bir_lowering=False)

    x_ext = nc.declare_dram_parameter("att_input", [S, D], F32, isOutput=False)
    wq_ext = nc.declare_dram_parameter("Wq", [D, D], F32, isOutput=False)
    wk_ext = nc.declare_dram_parameter("Wk", [D, D], F32, isOutput=False)
    wv_ext = nc.declare_dram_parameter("Wv", [D, D], F32, isOutput=False)
    out_ext = nc.declare_dram_parameter("out", [S, D], F32, isOutput=True)

    x_view = x_ext[:].bitcast(F32R).rearrange("(t p) d -> p t d", p=P)
    out_view = out_ext[:].rearrange("(c s p) d -> c p s d", s=Q_CHUNK // P, p=P)

    XCH = 8
    XSTRIDE = N_KTILES // XCH

    def group_kts(g):
        return list(range(KT_GRP * g, min(KT_GRP * g + KT_GRP, N_KTILES)))

    with TileContext(nc) as tc:
        with (
            tc.tile_pool(name="const", bufs=1) as cpool,
            tc.tile_pool(name="p1sb", bufs=2) as p1sb,
            # three units of exp tiles: decouples the exp streams from
            # the PV accumulators (PV re-reads each tile twice, in 2-sub
            # waves, so the accumulators need only 2 PSUM banks) and lets
            # both tail half-units pre-emit together
            tc.tile_pool(name="expp", bufs=3 * N_KG) as epool,
            tc.tile_pool(name="outp", bufs=4) as opool,
            tc.tile_pool(name="nrm", bufs=4) as npool,
            # scores pool: (8-2 banks)/KT_GRP tiles, disjoint from the
            # phase-1 pool (2-deep x 3 banks, or 3-deep x 2 banks)
            tc.tile_pool(name="ps_s", bufs=6 // KT_GRP, space="PSUM") as ps_s,
        ):
            # Identity is built as f32 (gpsimd memset rejects f32r) and
            # bitcast to f32r at the transpose sites: transpose cost
            # follows the MOVING operand (the identity), and f32r
            # transposes at 1.5 cyc/row vs f32's 2.0.  bf16 would be 1.0
            # but the walrus verifier rejects mixed 32/16-bit matmul
            # inputs (NCC_IBIR034).
            ident = cpool.tile([P, P], F32)
            ident_r = cpool.tile([P, P], F32R)  # rounded copy for PE
            xT = cpool.tile([P, S], F32R)  # [d, s]
            xTb = cpool.tile([P, S], BF16)  # bf16 copy for V-projection
            m_sb = cpool.tile([P, P], F32R)  # M[d, d'] = Wq^T @ Wk
            aT = cpool.tile([P, S], F32R)  # [d', s] = (x @ M)^T
            vones = cpool.tile([P, N_KTILES, 132], BF16)  # [k, t, e|1]
            wvb = cpool.tile([P, P], BF16)  # Wv^T in bf16
            negshift = cpool.tile([P, 1], F32)

            nc.vector.memset(vones[:, :, P : P + 1], 1.0)
            nc.vector.memset(negshift[:], -SHIFT)

            # DMAs: wq + wk (gate M), x in 8 chunks, wv last -- all on the
            # SP HWDGE ring (the DGE fixed latency dominates, so leading
            # with x0 or splitting rings does not pay; measured).
            make_identity(nc, ident)
            nc.vector.tensor_copy(ident_r[:], ident[:])
            w_nats = {}
            for nm, w_ext in (("wq", wq_ext), ("wk", wk_ext)):
                w_nat = p1sb.tile([P, P], F32, tag="wnat", name=f"wn_{nm}")
                nc.sync.dma_start(w_nat[:], w_ext[:])
                w_nats[nm] = w_nat
            x_sb = []
            for ci in range(XCH):
                xs = cpool.tile([P, XSTRIDE, P], F32R, name=f"x_sb{ci}")
                ring = (
                    nc.gpsimd
                    if X_RING == "pool_odd" and ci % 2 == 1
                    else nc.sync
                )
                ring.dma_start(
                    xs[:], x_view[:, ci * XSTRIDE : (ci + 1) * XSTRIDE]
                )
                x_sb.append(xs)
            wv_nat = p1sb.tile([P, P], F32R, tag="wnat", name="wn_wv")
            nc.sync.dma_start(wv_nat[:], wv_ext[:].bitcast(F32R))

            def scores_exp(q0, w, g, split_exp=False, dve=False):
                """scores + exp for one k-tile group over queries
                [q0, q0+w); returns the exp tile."""
                qs = slice(q0, q0 + w)
                kts = group_kts(g)
                n = len(kts)
                ps = ps_s.tile([P, KT_GRP, Q_CHUNK], F32, tag="ps")
                for j, kt in enumerate(kts):
                    nc.tensor.matmul(
                        ps[:, j, 0:w],
                        xT[:, kt * P : (kt + 1) * P],
                        aT[:, qs],
                        start=True,
                        stop=True,
                    )
                ex = epool.tile([P, KT_GRP, Q_CHUNK], BF16, tag="ex")
                if dve:
                    nc.vector.tensor_scalar(
                        ex[:, 0:n, 0:w].bitcast(I16),
                        ps[:, 0:n, 0:w],
                        float(C1),
                        0.0,
                        mybir.AluOpType.mult,
                        mybir.AluOpType.max,
                    )
                elif split_exp:
                    for j in range(n):
                        nc.scalar.activation(
                            ex[:, j, 0:w], ps[:, j, 0:w],
                            mybir.ActivationFunctionType.Exp,
                            bias=negshift[:],
                        )
                else:
                    nc.scalar.activation(
                        ex[:, 0:n, 0:w], ps[:, 0:n, 0:w],
                        mybir.ActivationFunctionType.Exp,
                        bias=negshift[:],
                    )
                return ex

            def pv_wave(po2, exs, subs):
                """PV for two unit-local q-subtiles over all k-tiles."""
                for kt in range(N_KTILES):
                    ex = exs[kt // KT_GRP]
                    j = kt % KT_GRP
                    for i, sub in enumerate(subs):
                        nc.tensor.matmul(
                            po2[i][:, 0 : P + 1],
                            ex[:, j, sub * P : (sub + 1) * P],
                            vones[:, kt, 0 : P + 1],
                            start=(kt == 0),
                            stop=(kt == N_KTILES - 1),
                        )

            def finish_wave(gsubs, po2, tail=False):
                """normalize + DMA for two GLOBAL q-subtile indices.  On
                the final wave the second subtile's multiply runs on the
                (by then idle) ACT engine and its DMA rides the ACT ring,
                so the two normalize+writeback chains run in parallel."""
                out_sb = opool.tile([P, 2, P], F32, tag="osb")
                for i, gs in enumerate(gsubs):
                    rec = npool.tile([P, 1], F32, tag="rec")
                    nc.vector.reciprocal(rec[:], po2[i][:, P : P + 1])
                    if tail and i == 1:
                        nc.scalar.mul(out_sb[:, i], po2[i][:, 0:P], rec[:])
                        nc.scalar.dma_start(
                            out_view[gs // 4, :, gs % 4], out_sb[:, i]
                        )
                    else:
                        nc.vector.tensor_scalar_mul(
                            out_sb[:, i], po2[i][:, 0:P], rec[:]
                        )
                        nc.sync.dma_start(
                            out_view[gs // 4, :, gs % 4], out_sb[:, i]
                        )

            # ---- phase 1 + chunk-0 scores/exps, interleaved with x arrival;
            # group g emitted once its k-tiles' xT groups have landed
            exs0 = []
            with tc.tile_pool(name="p1ps", bufs=2, space="PSUM") as p1ps:
                pm = p1ps.tile([P, 1, Q_CHUNK], F32, tag="p1", name="pm")
                nc.tensor.matmul(
                    pm[:, 0, 0:P], w_nats["wq"][:], w_nats["wk"][:],
                    start=True, stop=True,
                )
                nc.scalar.copy(m_sb[:], pm[:, 0, 0:P])

                def xpose_group(g):
                    pt = p1ps.tile([P, 1, Q_CHUNK], F32, tag="p1", name=f"pt{g}")
                    ptv = pt[:, 0].rearrange("p (a b) -> p a b", b=P)
                    for j in range(4):
                        t = 4 * g + j
                        nc.tensor.transpose(
                            ptv[:, j].bitcast(F32R),
                            x_sb[t // XSTRIDE][:, t % XSTRIDE],
                            ident_r[:],
                        )
                    nc.vector.tensor_copy(
                        xT[:, g * 512 : (g + 1) * 512], pt[:, 0]
                    )

                def at_chunk(c):
                    pq = p1ps.tile([P, 1, Q_CHUNK], F32, tag="p1", name=f"pa{c}")
                    nc.tensor.matmul(
                        pq[:, 0],
                        m_sb[:],
                        xT[:, c * Q_CHUNK : (c + 1) * Q_CHUNK],
                        start=True,
                        stop=True,
                    )
                    (nc.scalar.copy if c == 0 else nc.vector.tensor_copy)(
                        aT[:, c * Q_CHUNK : (c + 1) * Q_CHUNK], pq[:, 0]
                    )

                next_g = 0
                for ci in range(XCH):
                    xpose_group(ci)
                    if ci == 0:
                        at_chunk(0)
                    # groups whose k-tiles are now transposed
                    while next_g < N_KG and (
                        group_kts(next_g)[-1] <= 4 * ci + 3
                    ):
                        exs0.append(
                            scores_exp(0, Q_CHUNK, next_g,
                                       dve=next_g in DVE_FULL)
                        )
                        next_g += 1
                at_chunk(1)

                # trailing phase-1 (off the critical path; DVE copies):
                # wv transpose, bf16 V projection, remaining aT chunks
                pw = p1ps.tile([P, 1, Q_CHUNK], F32, tag="p1", name="pw")
                nc.tensor.transpose(
                    pw[:, 0, 0:P].bitcast(F32R), wv_nat[:], ident_r[:]
                )
                nc.vector.tensor_copy(wvb[:], pw[:, 0, 0:P])
                for c in range(XCH):
                    nc.vector.tensor_copy(
                        xTb[:, c * 512 : (c + 1) * 512],
                        xT[:, c * 512 : (c + 1) * 512],
                    )
                for g in range(8):
                    pv = p1ps.tile([P, 1, Q_CHUNK], F32, tag="p1", name=f"pv{g}")
                    pvv = pv[:, 0].rearrange("p (a b) -> p a b", b=P)
                    for j in range(4):
                        t = 4 * g + j
                        nc.tensor.matmul(
                            pvv[:, j],
                            xTb[:, t * P : (t + 1) * P],
                            wvb[:],
                            start=True,
                            stop=True,
                        )
                    nc.vector.tensor_copy(
                        vones[:, 4 * g : 4 * g + 4, 0:P], pvv[:]
                    )
                for c in range(2, N_QCHUNKS):
                    at_chunk(c)

                # chunk-1 scores+exps pre-emitted (pipeline depth 1)
                exs1 = [
                    scores_exp(Q_CHUNK, Q_CHUNK, g, dve=g in DVE_FULL)
                    for g in range(N_KG)
                ]

            # ---- PV accumulators on the freed phase-1 banks (2): two
            # 2-subtile waves per chunk re-reading the buffered exp tiles
            with tc.tile_pool(name="ps_o", bufs=2, space="PSUM") as ps_o:
                # units: 7 full 512-wide chunks (two PV waves each) + two
                # 256-wide half-chunks at the end (ONE wave each, so the
                # final unit's PV trails its exps directly -- short tail)
                units = [(c * Q_CHUNK, Q_CHUNK) for c in range(7)]
                units += [(7 * Q_CHUNK, 256), (7 * Q_CHUNK + 256, 256)]
                exs = {0: exs0, 1: exs1}
                tail_la = _os.environ.get("ATT_TAIL_LA", "0") == "1"
                for u, (q0, w) in enumerate(units):
                    # pre-emit scores+exps one unit ahead (optionally both
                    # tail half-units together)
                    lookahead = (u + 1, u + 2) if tail_la else (u + 1,)
                    for nxt in lookahead:
                        if nxt >= len(units) or nxt in exs:
                            continue
                        nq0, nw = units[nxt]
                        if nxt == u + 2 and (
                            nw == Q_CHUNK or units[u + 1][1] == Q_CHUNK
                        ):
                            continue  # depth 2 only within the tail
                        last = nxt == len(units) - 1
                        dve_set = DVE_FULL if nw == Q_CHUNK else DVE_HALF
                        exs[nxt] = [
                            scores_exp(
                                nq0, nw, g,
                                split_exp=last and g == N_KG - 1,
                                dve=g in dve_set,
                            )
                            for g in range(N_KG)
                        ]
                    nsub = w // P
                    for wave in range(nsub // 2):
                        po2 = [
                            ps_o.tile([P, P + 1], F32, tag="po",
                                      name=f"po_{u}_{wave}_{i}")
                            for i in range(2)
                        ]
                        subs = (2 * wave, 2 * wave + 1)
                        pv_wave(po2, exs[u], subs)
                        finish_wave(
                            tuple(q0 // P + s for s in subs), po2,
                            tail=(u == len(units) - 1),
                        )
                    del exs[u]

    nc.compile()
    return nc


_NC_CACHE = {}


def _get_nc():
    if "nc" not in _NC_CACHE:
        _NC_CACHE["nc"] = build_attention_nc()
    return _NC_CACHE["nc"]


def _in_maps(att_input, Wq, Wk, Wv):
    att_input = np.ascontiguousarray(att_input, dtype=np.float32)
    Wq = np.ascontiguousarray(Wq, dtype=np.float32)
    Wk = np.ascontiguousarray(Wk, dtype=np.float32)
    Wv = np.ascontiguousarray(Wv, dtype=np.float32)
    return [
        {"att_input": att_input[b], "Wq": Wq, "Wk": Wk, "Wv": Wv}
        for b in range(N_CORES)
    ]


def _get_runner():
    """Build the 8-core jitted executable ONCE (jax.jit retrace per call is
    expensive); subsequent kernel() calls reuse it."""
    if "runner" in _NC_CACHE:
        return _NC_CACHE["runner"]

    import jax
    from jax.sharding import Mesh, PartitionSpec
    from jax.experimental.shard_map import shard_map
    from concourse import bass2jax

    nc = _get_nc()
    bass2jax.install_neuronx_cc_hook()
    partition_name = nc.partition_id_tensor.name if nc.partition_id_tensor else None

    in_names, out_names, out_avals, zero_shapes = [], [], [], []
    for alloc in nc.m.functions[0].allocations:
        if not isinstance(alloc, mybir.MemoryLocationSet):
            continue
        name = alloc.memorylocations[0].name
        if alloc.kind == "ExternalInput":
            if name != partition_name:
                in_names.append(name)
        elif alloc.kind == "ExternalOutput":
            out_names.append(name)
            shape = tuple(alloc.tensor_shape)
            dtype = mybir.dt.np(alloc.dtype)
            out_avals.append(jax.core.ShapedArray(shape, dtype))
            zero_shapes.append((shape, dtype))
    n_params = len(in_names)
    all_in_names = list(in_names) + list(out_names)
    if partition_name is not None:
        all_in_names.append(partition_name)

    def _body(*args):
        operands = list(args)
        if partition_name is not None:
            operands.append(bass2jax.partition_id_tensor())
        outs = bass2jax._bass_exec_p.bind(
            *operands,
            out_avals=tuple(out_avals),
            in_names=tuple(all_in_names),
            out_names=tuple(out_names),
            lowering_input_output_aliases=(),
            sim_require_finite=True,
            sim_require_nnan=True,
            nc=nc,
        )
        return tuple(outs)

    devices = jax.devices()[:N_CORES]
    mesh = Mesh(np.asarray(devices), ("core",))
    in_specs = (PartitionSpec("core"),) * (n_params + len(out_names))
    out_specs = (PartitionSpec("core"),) * len(out_names)
    fn = jax.jit(
        shard_map(_body, mesh=mesh, in_specs=in_specs, out_specs=out_specs,
                  check_rep=False),
        keep_unused=True,
    )
    _NC_CACHE["runner"] = (fn, in_names, zero_shapes)
    return _NC_CACHE["runner"]


def kernel(att_input, Wq, Wk, Wv):
    fn, in_names, zero_shapes = _get_runner()
    in_maps = _in_maps(att_input, Wq, Wk, Wv)
    concat_in = [
        np.concatenate([in_maps[c][name] for c in range(N_CORES)], axis=0)
        for name in in_names
    ]
    concat_zeros = [
        np.zeros((N_CORES * shape[0], *shape[1:]), dtype)
        for shape, dtype in zero_shapes
    ]
    outs = fn(*concat_in, *concat_zeros)
    out = np.asarray(outs[0]).reshape(N_CORES, S, D)
    return out


def kernel_via_spmd(att_input, Wq, Wk, Wv):
    """Reference path through run_bass_kernel_spmd (slower per call)."""
    nc = _get_nc()
    res = run_bass_kernel_spmd(
        nc, _in_maps(att_input, Wq, Wk, Wv), core_ids=list(range(N_CORES))
    )
    return np.stack([res.results[b]["out"] for b in range(N_CORES)], axis=0)
